# revision 1
# baseline (speedup 1.0000x reference)
"""AtomPlacementScheduler Trainium2 kernel.

out[b] = sum_e irfft(rfft(stems[b,e]) * exp(-2i pi f s_be)),  s = sigmoid(TL@W+b)*N.

Implemented as a 4-step FFT (N = 32768 = 128 x 256) so all heavy work is
TensorEngine matmuls; the per-event shift phase factors into A[k2] (folded into
the stage-2 twiddle multiply) and B[k1] (folded into the per-event stage-3 DFT
matrix), and the sum over 16 events is free PSUM accumulation.  Pure data
parallel over batch: 64 batches / 8 cores = 8 per core.

Self-contained: hardcodes shapes B=64, E=16, N=32768, n_cores=8.
"""
import numpy as np
import ml_dtypes

N = 32768
N1 = 128   # stage-3 DFT size
N2 = 256   # stage-1 DFT size
E = 16
B = 64
NCORES = 8
BC = B // NCORES      # 8 batches per core
S = BC * E            # 128 signals per core
K1 = 65               # k1 = 0..64 covers k = k2 + 256*k1 up to Nyquist

F32 = np.float32
BF16 = ml_dtypes.bfloat16


def _host_tables():
    n1 = np.arange(N1)
    n2 = np.arange(N2)
    k2 = np.arange(N2)
    k1 = np.arange(K1)
    W2 = np.exp(-2j * np.pi * np.outer(n2, k2) / N2)        # (n2, k2)
    W2cat = np.concatenate([W2.real, W2.imag], 1)           # (256, 512)
    T = np.exp(-2j * np.pi * np.outer(n1, k2) / N)          # (n1, k2)
    W1 = np.exp(-2j * np.pi * np.outer(n1, k1) / N1)        # (n1, k1)
    E1 = np.exp(+2j * np.pi * np.outer(np.arange(64), np.arange(N1)) / N1)  # (k1, m)
    Tinv = np.exp(+2j * np.pi * np.outer(np.arange(N1), k2) / N)            # (m, k2)
    E2 = np.exp(+2j * np.pi * np.outer(k2, np.arange(N2)) / N2) * (2.0 / N)  # (k2, j)
    return W2cat, T, W1, E1, Tinv, E2


def _build_graph():
    import concourse.bass as bass
    import concourse.mybir as mybir
    import concourse.tile as tile
    from concourse import bacc

    dt = mybir.dt
    nc = bacc.Bacc("TRN2", target_bir_lowering=False, debug=False, num_devices=NCORES)

    # ---- DRAM parameters (per-core shard shapes) ----
    stems_d = nc.dram_tensor("stems", [BC, E, N2, N1], dt.float32, kind="ExternalInput")
    a_d = nc.dram_tensor("a_tab", [1, S * 512], dt.bfloat16, kind="ExternalInput")
    b_d = nc.dram_tensor("b_tab", [1, S * 130], dt.bfloat16, kind="ExternalInput")
    w2_d = nc.dram_tensor("w2cat", [N2, 512], dt.bfloat16, kind="ExternalInput")
    tc_d = nc.dram_tensor("t_c", [N1, N2], dt.bfloat16, kind="ExternalInput")
    ts_d = nc.dram_tensor("t_s", [N1, N2], dt.bfloat16, kind="ExternalInput")
    w1_d = nc.dram_tensor("w1cs", [N1, 130], dt.bfloat16, kind="ExternalInput")   # [W1c|W1s]
    e1c_d = nc.dram_tensor("e1c", [64, N1], dt.bfloat16, kind="ExternalInput")
    e1s_d = nc.dram_tensor("e1s", [64, N1], dt.bfloat16, kind="ExternalInput")
    tic_d = nc.dram_tensor("ti_c", [N1, N2], dt.bfloat16, kind="ExternalInput")
    tis_d = nc.dram_tensor("ti_s", [N1, N2], dt.bfloat16, kind="ExternalInput")
    e2c_d = nc.dram_tensor("e2c", [N2, N2], dt.bfloat16, kind="ExternalInput")
    e2sn_d = nc.dram_tensor("e2sn", [N2, N2], dt.bfloat16, kind="ExternalInput")
    ones_d = nc.dram_tensor("ones", [1, 128], dt.bfloat16, kind="ExternalInput")
    out_d = nc.dram_tensor("out", [BC, N2, N1], dt.float32, kind="ExternalOutput")
    aux_d = nc.dram_tensor("aux", [BC, 2], dt.float32, kind="ExternalOutput")

    with tile.TileContext(nc) as tc:
        with (
            tc.tile_pool(name="const", bufs=1) as cpool,
            tc.tile_pool(name="work", bufs=3) as pool,
            tc.tile_pool(name="psum", bufs=2, space="PSUM") as psum,
            tc.tile_pool(name="psacc", bufs=1, space="PSUM") as psacc,
        ):
            # ---- load constants once ----
            w2_0 = cpool.tile([128, 512], dt.bfloat16, tag="w2_0")
            w2_1 = cpool.tile([128, 512], dt.bfloat16, tag="w2_1")
            w2 = [w2_0, w2_1]
            nc.sync.dma_start(w2[0][:], w2_d[0:128, :])
            nc.sync.dma_start(w2[1][:], w2_d[128:256, :])
            t_c = cpool.tile([N1, N2], dt.bfloat16, tag="tc")
            t_s = cpool.tile([N1, N2], dt.bfloat16, tag="ts")
            nc.sync.dma_start(t_c[:], tc_d[:])
            nc.sync.dma_start(t_s[:], ts_d[:])
            w1 = cpool.tile([N1, 130], dt.bfloat16, tag="w1")
            nc.sync.dma_start(w1[:], w1_d[:])
            a_sb = cpool.tile([1, S * 512], dt.bfloat16, tag="a")
            nc.sync.dma_start(a_sb[:], a_d[:])
            b_sb = cpool.tile([1, S * 130], dt.bfloat16, tag="b")
            nc.sync.dma_start(b_sb[:], b_d[:])
            e1c = cpool.tile([64, N1], dt.bfloat16, tag="e1c")
            e1s = cpool.tile([64, N1], dt.bfloat16, tag="e1s")
            nc.sync.dma_start(e1c[:], e1c_d[:])
            nc.sync.dma_start(e1s[:], e1s_d[:])
            ti_c = cpool.tile([N1, N2], dt.bfloat16, tag="tic")
            ti_s = cpool.tile([N1, N2], dt.bfloat16, tag="tis")
            nc.sync.dma_start(ti_c[:], tic_d[:])
            nc.sync.dma_start(ti_s[:], tis_d[:])
            e2c_0 = cpool.tile([128, N2], dt.bfloat16, tag="e2c_0")
            e2c_1 = cpool.tile([128, N2], dt.bfloat16, tag="e2c_1")
            e2sn_0 = cpool.tile([128, N2], dt.bfloat16, tag="e2sn_0")
            e2sn_1 = cpool.tile([128, N2], dt.bfloat16, tag="e2sn_1")
            e2c = [e2c_0, e2c_1]
            e2sn = [e2sn_0, e2sn_1]
            nc.sync.dma_start(e2c[0][:], e2c_d[0:128, :])
            nc.sync.dma_start(e2c[1][:], e2c_d[128:256, :])
            nc.sync.dma_start(e2sn[0][:], e2sn_d[0:128, :])
            nc.sync.dma_start(e2sn[1][:], e2sn_d[128:256, :])
            ones = cpool.tile([1, 128], dt.bfloat16, tag="ones")
            nc.sync.dma_start(ones[:], ones_d[:])

            for b in range(BC):
                pA = psacc.tile([K1, 512], dt.float32, tag="pA")
                pB = psacc.tile([K1, 512], dt.float32, tag="pB")
                for e in range(E):
                    sig = b * E + e
                    # stage 1: xm chunks (cast f32->bf16 via gpsimd dma)
                    xm0 = pool.tile([128, N1], dt.bfloat16, tag="xm0")
                    xm1 = pool.tile([128, N1], dt.bfloat16, tag="xm1")
                    nc.gpsimd.dma_start(xm0[:], stems_d[b, e, 0:128, :])
                    nc.gpsimd.dma_start(xm1[:], stems_d[b, e, 128:256, :])
                    p1 = psum.tile([N1, 512], dt.float32, tag="p1")
                    nc.tensor.matmul(p1[:], xm0[:], w2[0][:], start=True, stop=False)
                    nc.tensor.matmul(p1[:], xm1[:], w2[1][:], start=False, stop=True)
                    # broadcast A and B rows across partitions via ones-matmul
                    pab = psum.tile([128, 512], dt.float32, tag="pbc")
                    nc.tensor.matmul(pab[:], ones[:],
                                     a_sb[0:1, sig * 512 : sig * 512 + 512],
                                     start=True, stop=True)
                    pbb = psum.tile([128, 512], dt.float32, tag="pbc")
                    nc.tensor.matmul(pbb[:, 0:130], ones[:],
                                     b_sb[0:1, sig * 130 : sig * 130 + 130],
                                     start=True, stop=True)
                    ab = pool.tile([128, 512], dt.bfloat16, tag="ab")
                    nc.any.tensor_copy(ab[:], pab[:])
                    bb = pool.tile([128, 130], dt.bfloat16, tag="bb")
                    nc.any.tensor_copy(bb[:], pbb[:, 0:130])
                    # C = T * A  (complex), Cc/Cs (128,256) bf16
                    cc = pool.tile([N1, N2], dt.bfloat16, tag="cc")
                    cs = pool.tile([N1, N2], dt.bfloat16, tag="cs")
                    tmp1 = pool.tile([N1, N2], dt.bfloat16, tag="tmp1")
                    tmp2 = pool.tile([N1, N2], dt.bfloat16, tag="tmp2")
                    nc.any.tensor_mul(tmp1[:], t_c[:], ab[:, 0:256])
                    nc.any.tensor_mul(tmp2[:], t_s[:], ab[:, 256:512])
                    nc.any.tensor_sub(cc[:], tmp1[:], tmp2[:])
                    nc.any.tensor_mul(tmp1[:], t_c[:], ab[:, 256:512])
                    nc.any.tensor_mul(tmp2[:], t_s[:], ab[:, 0:256])
                    nc.any.tensor_add(cs[:], tmp1[:], tmp2[:])
                    # UV = inner * C: U = Pre*Cc - Pim*Cs ; V = Pre*Cs + Pim*Cc
                    uv = pool.tile([N1, 512], dt.bfloat16, tag="uv")
                    nc.any.tensor_mul(tmp1[:], p1[:, 0:256], cc[:])
                    nc.any.tensor_mul(tmp2[:], p1[:, 256:512], cs[:])
                    nc.any.tensor_sub(uv[:, 0:256], tmp1[:], tmp2[:])
                    nc.any.tensor_mul(tmp1[:], p1[:, 0:256], cs[:])
                    nc.any.tensor_mul(tmp2[:], p1[:, 256:512], cc[:])
                    nc.any.tensor_add(uv[:, 256:512], tmp1[:], tmp2[:])
                    # M = W1 * B (complex), (128, 65) each
                    m_re = pool.tile([N1, K1], dt.bfloat16, tag="mre")
                    m_im = pool.tile([N1, K1], dt.bfloat16, tag="mim")
                    st1 = pool.tile([N1, K1], dt.bfloat16, tag="st1")
                    st2 = pool.tile([N1, K1], dt.bfloat16, tag="st2")
                    nc.any.tensor_mul(st1[:], w1[:, 0:65], bb[:, 0:65])
                    nc.any.tensor_mul(st2[:], w1[:, 65:130], bb[:, 65:130])
                    nc.any.tensor_sub(m_re[:], st1[:], st2[:])
                    nc.any.tensor_mul(st1[:], w1[:, 0:65], bb[:, 65:130])
                    nc.any.tensor_mul(st2[:], w1[:, 65:130], bb[:, 0:65])
                    nc.any.tensor_add(m_im[:], st1[:], st2[:])
                    # stage 3, accumulating over events
                    nc.tensor.matmul(pA[:], m_re[:], uv[:],
                                     start=(e == 0), stop=(e == E - 1))
                    nc.tensor.matmul(pB[:], m_im[:], uv[:],
                                     start=(e == 0), stop=(e == E - 1))
                # combine into X (65, 512) bf16: [Xre|Xim]
                xf = pool.tile([K1, 512], dt.bfloat16, tag="xf")
                pbsb = pool.tile([K1, 512], dt.bfloat16, tag="pbsb")
                nc.any.tensor_copy(pbsb[:], pB[:])
                nc.any.tensor_sub(xf[:, 0:256], pA[:, 0:256], pbsb[:, 256:512])
                nc.any.tensor_add(xf[:, 256:512], pA[:, 256:512], pbsb[:, 0:256])
                nc.gpsimd.dma_start(aux_d[b, 0:1], xf[0:1, 0:1])
                nc.gpsimd.dma_start(aux_d[b, 1:2], xf[64:65, 0:1])
                # inverse stage I1: G = E1^T @ X   (contract k1=64)
                pga = psum.tile([N1, 512], dt.float32, tag="pinv")
                pgb = psum.tile([N1, 512], dt.float32, tag="pinv")
                nc.tensor.matmul(pga[:], e1c[:], xf[0:64, :], start=True, stop=True)
                nc.tensor.matmul(pgb[:], e1s[:], xf[0:64, :], start=True, stop=True)
                g_re = pool.tile([N1, N2], dt.bfloat16, tag="gre")
                g_im = pool.tile([N1, N2], dt.bfloat16, tag="gim")
                gbsb = pool.tile([N1, 512], dt.bfloat16, tag="gbsb")
                nc.any.tensor_copy(gbsb[:], pgb[:])
                nc.any.tensor_sub(g_re[:], pga[:, 0:256], gbsb[:, 256:512])
                nc.any.tensor_add(g_im[:], pga[:, 256:512], gbsb[:, 0:256])
                # twiddle: GT = G * Tinv
                gt_re = pool.tile([N1, N2], dt.bfloat16, tag="gtre")
                gt_im = pool.tile([N1, N2], dt.bfloat16, tag="gtim")
                it1 = pool.tile([N1, N2], dt.bfloat16, tag="it1")
                it2 = pool.tile([N1, N2], dt.bfloat16, tag="it2")
                nc.any.tensor_mul(it1[:], g_re[:], ti_c[:])
                nc.any.tensor_mul(it2[:], g_im[:], ti_s[:])
                nc.any.tensor_sub(gt_re[:], it1[:], it2[:])
                nc.any.tensor_mul(it1[:], g_re[:], ti_s[:])
                nc.any.tensor_mul(it2[:], g_im[:], ti_c[:])
                nc.any.tensor_add(gt_im[:], it1[:], it2[:])
                # transpose GT -> (k2, m), 2 chunks each
                gtt_re_0 = pool.tile([128, N1], dt.bfloat16, tag="gttre0")
                gtt_re_1 = pool.tile([128, N1], dt.bfloat16, tag="gttre1")
                gtt_im_0 = pool.tile([128, N1], dt.bfloat16, tag="gttim0")
                gtt_im_1 = pool.tile([128, N1], dt.bfloat16, tag="gttim1")
                gtt_re = [gtt_re_0, gtt_re_1]
                gtt_im = [gtt_im_0, gtt_im_1]
                for kc in range(2):
                    nc.sync.dma_start_transpose(
                        gtt_re[kc][:], gt_re[:, 128 * kc : 128 * kc + 128])
                    nc.sync.dma_start_transpose(
                        gtt_im[kc][:], gt_im[:, 128 * kc : 128 * kc + 128])
                # I4: S[j, m] = sum_k2 E2[k2,j] GTt[k2,m]  (real part only)
                for jc in range(2):
                    ps = psum.tile([128, 512], dt.float32, tag="pinv")
                    for kc in range(2):
                        nc.tensor.matmul(
                            ps[:, 0:128], e2c[kc][:, 128 * jc : 128 * jc + 128],
                            gtt_re[kc][:],
                            start=(kc == 0), stop=False)
                        nc.tensor.matmul(
                            ps[:, 0:128], e2sn[kc][:, 128 * jc : 128 * jc + 128],
                            gtt_im[kc][:],
                            start=False, stop=(kc == 1))
                    y_sb = pool.tile([128, N1], dt.float32, tag="ysb")
                    nc.any.tensor_copy(y_sb[:], ps[:, 0:128])
                    nc.sync.dma_start(out_d[b, 128 * jc : 128 * jc + 128, :], y_sb[:])
    nc.compile()
    return nc


def kernel(time_latent, stems, targets, W_pos, b_pos):
    from concourse.bass_utils import run_bass_kernel_spmd

    # host: positions (tiny linear+sigmoid, fp32 exactly like the reference)
    z = np.einsum("bed,od->beo", time_latent.astype(F32), W_pos.astype(F32))
    z = z.reshape(B, E) + b_pos.reshape(1)[0]
    pos = 1.0 / (1.0 + np.exp(-z, dtype=F32))
    s = pos * np.float32(N)

    W2cat, T, W1, E1, Tinv, E2 = _host_tables()
    k2 = np.arange(N2)
    k1 = np.arange(K1)

    nc = _build_graph()
    in_maps = []
    for c in range(NCORES):
        sl = slice(c * BC, (c + 1) * BC)
        s_flat = s[sl].reshape(-1).astype(np.float64)
        A = np.exp(-2j * np.pi * np.outer(s_flat, k2) / N)
        Bt = np.exp(-2j * np.pi * np.outer(s_flat, k1) / N1)
        in_maps.append({
            "stems": np.ascontiguousarray(stems[sl]).reshape(BC, E, N2, N1),
            "a_tab": np.concatenate([A.real, A.imag], 1).astype(BF16).reshape(1, -1),
            "b_tab": np.concatenate([Bt.real, Bt.imag], 1).astype(BF16).reshape(1, -1),
            "w2cat": W2cat.astype(BF16),
            "t_c": T.real.astype(BF16),
            "t_s": T.imag.astype(BF16),
            "w1cs": np.concatenate([W1.real, W1.imag], 1).astype(BF16),
            "e1c": E1.real.astype(BF16),
            "e1s": E1.imag.astype(BF16),
            "ti_c": Tinv.real.astype(BF16),
            "ti_s": Tinv.imag.astype(BF16),
            "e2c": E2.real.astype(BF16),
            "e2sn": (-E2.imag).astype(BF16),
            "ones": np.ones((1, 128), dtype=BF16),
        })

    import os
    trace = bool(int(os.environ.get("ATHENA_TRACE", "0")))
    res = run_bass_kernel_spmd(nc, in_maps, core_ids=list(range(NCORES)), trace=trace)
    if trace:
        print(f"HW exec time: {res.exec_time_ns} ns")
    outs = []
    sign = np.where(np.arange(N) % 2 == 0, 1.0, -1.0).astype(F32)
    for c in range(NCORES):
        y = res.results[c]["out"].reshape(BC, N).astype(F32)
        aux = res.results[c]["aux"].astype(F32)          # (BC, 2) = X0, XN2re
        y = y + (-aux[:, 0:1] + sign[None, :] * aux[:, 1:2]) / np.float32(N)
        outs.append(y)
    return np.concatenate(outs, 0).reshape(B, 1, N).astype(F32)



# revision 6
# speedup vs baseline: 2.0953x; 2.0953x over previous
"""AtomPlacementScheduler Trainium2 kernel.

out[b] = sum_e irfft(rfft(stems[b,e]) * exp(-2i pi f s_be)),  s = sigmoid(TL@W+b)*N.

4-step FFT (N = 32768 = 256 x 128): all heavy work is TensorEngine matmuls.
Host ships, per event, a packed bf16 record [stems | Cre | Cim | Mre | Mim | -Mim]
where C = twiddle*shift-phase (n1 x k2) and M = W1*diag(B) (n1 x k1), so the
device does: stage-1 DFT (2 matmuls), one PSUM->SBUF cast, one complex
elementwise multiply (6 ops split vector/gpsimd), stage-3 DFT (4 matmuls
accumulating X re/im over the 16 events in PSUM).  Per batch, a transpose-free
inverse FFT: chunked matmuls produce G in transposed layout directly, twiddle,
then the final inner inverse DFT with real-part-only output.

Pure data parallel over batch: 64 batches / 8 cores = 8 per core.
Self-contained: hardcodes shapes B=64, E=16, N=32768, n_cores=8.
"""
import numpy as np
import ml_dtypes

N = 32768
N1 = 128   # outer DFT size (n1, k1)
N2 = 256   # inner DFT size (n2, k2)
E = 16
B = 64
NCORES = 8
BC = B // NCORES      # 8 batches per core
S = BC * E            # 128 signals per core
K1 = 65               # k1 = 0..64 covers k = k2 + 256*k1 up to Nyquist
RECW = 964            # 256 stems | 256 Cre | 256 Cim | 65 Mre | 65 Mim | 65 -Mim | 1 pad

F32 = np.float32
BF16 = ml_dtypes.bfloat16


def _host_consts():
    n1 = np.arange(N1)
    n2 = np.arange(N2)
    k2 = np.arange(N2)
    k1 = np.arange(K1)
    W2 = np.exp(-2j * np.pi * np.outer(n2, k2) / N2)            # (n2, k2)
    W2cat = np.concatenate([W2.real, W2.imag], 1)               # (256, 512)
    E1 = np.exp(+2j * np.pi * np.outer(k1[:64], n1) / N1)       # (k1<64, m)
    e1cat = np.zeros((K1, 384))
    e1cat[:64, 0:128] = E1.real
    e1cat[:64, 128:256] = E1.imag
    e1cat[:64, 256:384] = -E1.imag
    TinvT = np.exp(+2j * np.pi * np.outer(k2, n1) / N)          # (k2, m)
    tinv = np.zeros((2, 128, 256))
    for c in range(2):
        tinv[c, :, 0:128] = TinvT.real[c * 128:(c + 1) * 128]
        tinv[c, :, 128:256] = TinvT.imag[c * 128:(c + 1) * 128]
    E2 = np.exp(+2j * np.pi * np.outer(k2, n2) / N2) * (2.0 / N)  # (k2, n2)
    e2 = np.zeros((2, 128, 512))
    for c in range(2):
        e2[c, :, 0:256] = E2.real[c * 128:(c + 1) * 128]
        e2[c, :, 256:512] = -E2.imag[c * 128:(c + 1) * 128]
    return W2cat, e1cat, tinv, e2


def _build_graph():
    import concourse.bass as bass
    import concourse.mybir as mybir
    import concourse.tile as tile
    from concourse import bacc

    dt = mybir.dt
    nc = bacc.Bacc("TRN2", target_bir_lowering=False, debug=False, num_devices=NCORES)

    rec_d = nc.dram_tensor("rec", [BC, E, 128, RECW], dt.bfloat16, kind="ExternalInput")
    w2_d = nc.dram_tensor("w2cat", [N2, 512], dt.bfloat16, kind="ExternalInput")
    e1_d = nc.dram_tensor("e1cat", [K1, 384], dt.bfloat16, kind="ExternalInput")
    tinv_d = nc.dram_tensor("tinv", [2, 128, 256], dt.bfloat16, kind="ExternalInput")
    e2_d = nc.dram_tensor("e2", [2, 128, 512], dt.bfloat16, kind="ExternalInput")
    out_d = nc.dram_tensor("out", [BC, N2, N1], dt.float32, kind="ExternalOutput")
    aux_d = nc.dram_tensor("aux", [BC, 2], dt.float32, kind="ExternalOutput")

    LAG = 3

    with tile.TileContext(nc) as tc:
        with (
            tc.tile_pool(name="const", bufs=1) as cpool,
            tc.tile_pool(name="rec", bufs=LAG + 3) as recpool,
            tc.tile_pool(name="work", bufs=4) as pool,
            tc.tile_pool(name="inv", bufs=2) as ipool,
            tc.tile_pool(name="p1p", bufs=3, space="PSUM") as p1pool,
            tc.tile_pool(name="pxp", bufs=2, space="PSUM") as pxpool,
            tc.tile_pool(name="pgp", bufs=2, space="PSUM") as pgpool,
            tc.tile_pool(name="pyp", bufs=1, space="PSUM") as pypool,
        ):
            w2_0 = cpool.tile([128, 512], dt.bfloat16, tag="w2_0")
            w2_1 = cpool.tile([128, 512], dt.bfloat16, tag="w2_1")
            nc.sync.dma_start(w2_0[:], w2_d[0:128, :])
            nc.sync.dma_start(w2_1[:], w2_d[128:256, :])
            e1 = cpool.tile([K1, 384], dt.bfloat16, tag="e1")
            nc.sync.dma_start(e1[:], e1_d[:])
            tinv_0 = cpool.tile([128, 256], dt.bfloat16, tag="tinv0")
            tinv_1 = cpool.tile([128, 256], dt.bfloat16, tag="tinv1")
            nc.sync.dma_start(tinv_0[:], tinv_d[0])
            nc.sync.dma_start(tinv_1[:], tinv_d[1])
            e2_0 = cpool.tile([128, 512], dt.bfloat16, tag="e2_0")
            e2_1 = cpool.tile([128, 512], dt.bfloat16, tag="e2_1")
            nc.sync.dma_start(e2_0[:], e2_d[0])
            nc.sync.dma_start(e2_1[:], e2_d[1])
            tinv = [tinv_0, tinv_1]
            e2t = [e2_0, e2_1]
            w2 = [w2_0, w2_1]

            slots = {}

            def front(i):
                b, e = divmod(i, E)
                rec = recpool.tile([128, RECW], dt.bfloat16, tag="rec")
                nc.sync.dma_start(rec[:], rec_d[b, e])
                p1 = p1pool.tile([128, 512], dt.float32, tag="p1")
                nc.tensor.matmul(p1[:], rec[:, 0:128], w2[0][:], start=True, stop=False)
                nc.tensor.matmul(p1[:], rec[:, 128:256], w2[1][:], start=False, stop=True)
                slots[i] = (rec, p1)

            def back(i):
                b, e = divmod(i, E)
                rec, p1 = slots.pop(i)
                p1sb = pool.tile([128, 512], dt.bfloat16, tag="p1sb")
                nc.scalar.copy(p1sb[:], p1[:])
                t1 = pool.tile([128, 256], dt.bfloat16, tag="t1")
                t2 = pool.tile([128, 256], dt.bfloat16, tag="t2")
                t3 = pool.tile([128, 256], dt.bfloat16, tag="t3")
                t4 = pool.tile([128, 256], dt.bfloat16, tag="t4")
                uv = pool.tile([128, 512], dt.bfloat16, tag="uv")
                nc.vector.tensor_mul(t1[:], p1sb[:, 0:256], rec[:, 256:512])
                nc.vector.tensor_mul(t2[:], p1sb[:, 256:512], rec[:, 512:768])
                nc.vector.tensor_mul(t3[:], p1sb[:, 0:256], rec[:, 512:768])
                nc.gpsimd.tensor_mul(t4[:], p1sb[:, 256:512], rec[:, 256:512])
                nc.vector.tensor_sub(uv[:, 0:256], t1[:], t2[:])
                nc.vector.tensor_add(uv[:, 256:512], t3[:], t4[:])
                if e == 0:
                    slots[("pX", b)] = pxpool.tile([K1, 512], dt.float32, tag="pX",
                                                   name="pX")
                pX = slots[("pX", b)]
                # One accumulation group per PSUM bank: start only on the very
                # first matmul (start marks the whole 2KB zero region), stop on
                # the very last.
                st = e == 0
                sp = e == E - 1
                nc.tensor.matmul(pX[:, 0:256], rec[:, 768:833], uv[:, 0:256],
                                 start=st, stop=False)
                nc.tensor.matmul(pX[:, 0:256], rec[:, 898:963], uv[:, 256:512],
                                 start=False, stop=False)
                nc.tensor.matmul(pX[:, 256:512], rec[:, 768:833], uv[:, 256:512],
                                 start=False, stop=False)
                nc.tensor.matmul(pX[:, 256:512], rec[:, 833:898], uv[:, 0:256],
                                 start=False, stop=sp)
                if e == E - 1:
                    inverse(b, slots.pop(("pX", b)))

            def inverse(b, pX):
                xsb = ipool.tile([K1, 512], dt.bfloat16, tag="xsb")
                nc.scalar.copy(xsb[:], pX[:])
                nc.gpsimd.dma_start(aux_d[b, 0:1], xsb[0:1, 0:1])
                nc.gpsimd.dma_start(aux_d[b, 1:2], xsb[64:65, 0:1])
                pG = pgpool.tile([128, 512], dt.float32, tag="pG", name="pG")
                for c in range(2):
                    xre = xsb[:, c * 128:(c + 1) * 128]
                    xim = xsb[:, 256 + c * 128:256 + (c + 1) * 128]
                    o = c * 256
                    nc.tensor.matmul(pG[:, o:o + 128], xre, e1[:, 0:128],
                                     start=(c == 0), stop=False)
                    nc.tensor.matmul(pG[:, o:o + 128], xim, e1[:, 256:384],
                                     start=False, stop=False)
                    nc.tensor.matmul(pG[:, o + 128:o + 256], xre, e1[:, 128:256],
                                     start=False, stop=False)
                    nc.tensor.matmul(pG[:, o + 128:o + 256], xim, e1[:, 0:128],
                                     start=False, stop=(c == 1))
                gts = []
                for c in range(2):
                    gsb = ipool.tile([128, 256], dt.bfloat16, tag=f"gsb{c}")
                    nc.scalar.copy(gsb[:], pG[:, c * 256:(c + 1) * 256])
                    g1 = ipool.tile([128, 128], dt.bfloat16, tag=f"g1{c}")
                    g2 = ipool.tile([128, 128], dt.bfloat16, tag=f"g2{c}")
                    g3 = ipool.tile([128, 128], dt.bfloat16, tag=f"g3{c}")
                    g4 = ipool.tile([128, 128], dt.bfloat16, tag=f"g4{c}")
                    gt = ipool.tile([128, 256], dt.bfloat16, tag=f"gt{c}")
                    nc.vector.tensor_mul(g1[:], gsb[:, 0:128], tinv[c][:, 0:128])
                    nc.vector.tensor_mul(g2[:], gsb[:, 128:256], tinv[c][:, 128:256])
                    nc.vector.tensor_sub(gt[:, 0:128], g1[:], g2[:])
                    nc.gpsimd.tensor_mul(g3[:], gsb[:, 0:128], tinv[c][:, 128:256])
                    nc.gpsimd.tensor_mul(g4[:], gsb[:, 128:256], tinv[c][:, 0:128])
                    nc.vector.tensor_add(gt[:, 128:256], g3[:], g4[:])
                    gts.append(gt)
                pY = pypool.tile([128, 512], dt.float32, tag="pY", name="pY")
                for j in range(2):
                    nc.tensor.matmul(pY[:, j * 128:(j + 1) * 128],
                                     e2t[0][:, j * 128:(j + 1) * 128],
                                     gts[0][:, 0:128], start=(j == 0), stop=False)
                    nc.tensor.matmul(pY[:, j * 128:(j + 1) * 128],
                                     e2t[0][:, 256 + j * 128:256 + (j + 1) * 128],
                                     gts[0][:, 128:256], start=False, stop=False)
                    nc.tensor.matmul(pY[:, j * 128:(j + 1) * 128],
                                     e2t[1][:, j * 128:(j + 1) * 128],
                                     gts[1][:, 0:128], start=False, stop=False)
                    nc.tensor.matmul(pY[:, j * 128:(j + 1) * 128],
                                     e2t[1][:, 256 + j * 128:256 + (j + 1) * 128],
                                     gts[1][:, 128:256], start=False,
                                     stop=(j == 1))
                for j in range(2):
                    ysb = ipool.tile([128, 128], dt.float32, tag=f"ysb{j}")
                    nc.scalar.copy(ysb[:], pY[:, j * 128:(j + 1) * 128])
                    nc.sync.dma_start(out_d[b, j * 128:(j + 1) * 128, :], ysb[:])

            for i in range(S + LAG):
                if i < S:
                    front(i)
                if i >= LAG:
                    back(i - LAG)

    nc.compile()
    return nc


def kernel(time_latent, stems, targets, W_pos, b_pos):
    from concourse.bass_utils import run_bass_kernel_spmd

    # host: positions (tiny linear+sigmoid, fp32 exactly like the reference)
    z = np.einsum("bed,od->beo", time_latent.astype(F32), W_pos.astype(F32))
    z = z.reshape(B, E) + b_pos.reshape(1)[0]
    pos = 1.0 / (1.0 + np.exp(-z, dtype=F32))
    s = (pos * np.float32(N)).astype(np.float64)

    W2cat, e1cat, tinv, e2 = _host_consts()
    n1 = np.arange(N1)
    k2 = np.arange(N2)
    k1 = np.arange(K1)
    T = np.exp(-2j * np.pi * np.outer(n1, k2) / N)   # (n1, k2)
    W1 = np.exp(-2j * np.pi * np.outer(n1, k1) / N1)  # (n1, k1)

    w2cat_b = W2cat.astype(BF16)
    e1cat_b = e1cat.astype(BF16)
    tinv_b = tinv.astype(BF16)
    e2_b = e2.astype(BF16)

    nc = _build_graph()
    in_maps = []
    for c in range(NCORES):
        sl = slice(c * BC, (c + 1) * BC)
        s_flat = s[sl].reshape(-1)                                   # (S,)
        rec = np.empty((S, 128, RECW), dtype=BF16)
        # stems: (S, 256, 128) -> (S, 2, 128, 128) -> (S, 128, 2, 128)
        st = stems[sl].reshape(S, 2, 128, 128).transpose(0, 2, 1, 3)
        rec[:, :, 0:256] = st.reshape(S, 128, 256).astype(BF16)
        A = np.exp(-2j * np.pi * np.outer(s_flat, k2) / N)           # (S, k2)
        C = T[None, :, :] * A[:, None, :]                            # (S, n1, k2)
        rec[:, :, 256:512] = C.real.astype(BF16)
        rec[:, :, 512:768] = C.imag.astype(BF16)
        del C
        Bt = np.exp(-2j * np.pi * np.outer(s_flat, k1) / N1)         # (S, k1)
        M = W1[None, :, :] * Bt[:, None, :]                          # (S, n1, k1)
        rec[:, :, 768:833] = M.real.astype(BF16)
        rec[:, :, 833:898] = M.imag.astype(BF16)
        rec[:, :, 898:963] = (-M.imag).astype(BF16)
        rec[:, :, 963:964] = 0
        del M
        in_maps.append({
            "rec": rec.reshape(BC, E, 128, RECW),
            "w2cat": w2cat_b,
            "e1cat": e1cat_b,
            "tinv": tinv_b,
            "e2": e2_b,
        })

    import os
    trace = bool(int(os.environ.get("ATHENA_TRACE", "0")))
    res = run_bass_kernel_spmd(nc, in_maps, core_ids=list(range(NCORES)), trace=trace)
    if trace:
        print(f"HW exec time: {res.exec_time_ns} ns")
    outs = []
    sign = np.where(np.arange(N) % 2 == 0, 1.0, -1.0).astype(F32)
    for c in range(NCORES):
        y = res.results[c]["out"].reshape(BC, N).astype(F32)
        aux = res.results[c]["aux"].astype(F32)          # (BC, 2) = X0, XNyq
        y = y + (-aux[:, 0:1] + sign[None, :] * aux[:, 1:2]) / np.float32(N)
        outs.append(y)
    return np.concatenate(outs, 0).reshape(B, 1, N).astype(F32)


# revision 10
# speedup vs baseline: 2.3082x; 1.1016x over previous
"""AtomPlacementScheduler Trainium2 kernel.

out[b] = sum_e irfft(rfft(stems[b,e]) * exp(-2i pi f s_be)),  s = sigmoid(TL@W+b)*N.

4-step FFT (N = 32768 = 256 x 128): all heavy work is TensorEngine matmuls.
Host ships, per event, a packed bf16 record [stems | Cre | Cim | Mre | Mim | -Mim]
where C = twiddle*shift-phase (n1 x k2) and M = W1*diag(B) (n1 x k1), so the
device does: stage-1 DFT (2 matmuls), one PSUM->SBUF cast, one complex
elementwise multiply (6 ops split vector/gpsimd), stage-3 DFT (4 matmuls
accumulating X re/im over the 16 events in PSUM).  Per batch, a transpose-free
inverse FFT: chunked matmuls produce G in transposed layout directly, twiddle,
then the final inner inverse DFT with real-part-only output.

Pure data parallel over batch: 64 batches / 8 cores = 8 per core.
Self-contained: hardcodes shapes B=64, E=16, N=32768, n_cores=8.
"""
import numpy as np
import ml_dtypes

N = 32768
N1 = 128   # outer DFT size (n1, k1)
N2 = 256   # inner DFT size (n2, k2)
E = 16
B = 64
NCORES = 8
BC = B // NCORES      # 8 batches per core
S = BC * E            # 128 signals per core
K1 = 65               # k1 = 0..64 covers k = k2 + 256*k1 up to Nyquist
# 256 stems | 256 Cre | 256 Cim | 256 Cre (again, so [Cre|Cim] and [Cim|Cre]
# are both contiguous 512-wide slices) | 65 Mre | 65 Mim | 65 -Mim | 1 pad
RECW = 1220

F32 = np.float32
BF16 = ml_dtypes.bfloat16


def _host_consts():
    n1 = np.arange(N1)
    n2 = np.arange(N2)
    k2 = np.arange(N2)
    k1 = np.arange(K1)
    W2 = np.exp(-2j * np.pi * np.outer(n2, k2) / N2)            # (n2, k2)
    W2cat = np.concatenate([W2.real, W2.imag], 1)               # (256, 512)
    E1 = np.exp(+2j * np.pi * np.outer(k1[:64], n1) / N1)       # (k1<64, m)
    e1cat = np.zeros((K1, 384))
    e1cat[:64, 0:128] = E1.real
    e1cat[:64, 128:256] = E1.imag
    e1cat[:64, 256:384] = -E1.imag
    TinvT = np.exp(+2j * np.pi * np.outer(k2, n1) / N)          # (k2, m)
    tinv = np.zeros((2, 128, 256))
    for c in range(2):
        tinv[c, :, 0:128] = TinvT.real[c * 128:(c + 1) * 128]
        tinv[c, :, 128:256] = TinvT.imag[c * 128:(c + 1) * 128]
    E2 = np.exp(+2j * np.pi * np.outer(k2, n2) / N2) * (2.0 / N)  # (k2, n2)
    e2 = np.zeros((2, 128, 512))
    for c in range(2):
        e2[c, :, 0:256] = E2.real[c * 128:(c + 1) * 128]
        e2[c, :, 256:512] = -E2.imag[c * 128:(c + 1) * 128]
    return W2cat, e1cat, tinv, e2


def _build_graph():
    import concourse.bass as bass
    import concourse.mybir as mybir
    import concourse.tile as tile
    from concourse import bacc

    dt = mybir.dt
    nc = bacc.Bacc("TRN2", target_bir_lowering=False, debug=False, num_devices=NCORES)

    rec_d = nc.dram_tensor("rec", [BC, E, 128, RECW], dt.bfloat16, kind="ExternalInput")
    w2_d = nc.dram_tensor("w2cat", [N2, 512], dt.bfloat16, kind="ExternalInput")
    e1_d = nc.dram_tensor("e1cat", [K1, 384], dt.bfloat16, kind="ExternalInput")
    tinv_d = nc.dram_tensor("tinv", [2, 128, 256], dt.bfloat16, kind="ExternalInput")
    e2_d = nc.dram_tensor("e2", [2, 128, 512], dt.bfloat16, kind="ExternalInput")
    out_d = nc.dram_tensor("out", [BC, N2, N1], dt.float32, kind="ExternalOutput")
    aux_d = nc.dram_tensor("aux", [BC, 2], dt.float32, kind="ExternalOutput")

    LAG = 3

    with tile.TileContext(nc) as tc:
        with (
            tc.tile_pool(name="const", bufs=1) as cpool,
            tc.tile_pool(name="rec", bufs=LAG + 3) as recpool,
            tc.tile_pool(name="work", bufs=4) as pool,
            tc.tile_pool(name="inv", bufs=2) as ipool,
            tc.tile_pool(name="p1p", bufs=3, space="PSUM") as p1pool,
            tc.tile_pool(name="pxp", bufs=2, space="PSUM") as pxpool,
            tc.tile_pool(name="pgp", bufs=2, space="PSUM") as pgpool,
            tc.tile_pool(name="pyp", bufs=1, space="PSUM") as pypool,
        ):
            w2_0 = cpool.tile([128, 512], dt.bfloat16, tag="w2_0")
            w2_1 = cpool.tile([128, 512], dt.bfloat16, tag="w2_1")
            nc.sync.dma_start(w2_0[:], w2_d[0:128, :])
            nc.sync.dma_start(w2_1[:], w2_d[128:256, :])
            e1 = cpool.tile([K1, 384], dt.bfloat16, tag="e1")
            nc.sync.dma_start(e1[:], e1_d[:])
            tinv_0 = cpool.tile([128, 256], dt.bfloat16, tag="tinv0")
            tinv_1 = cpool.tile([128, 256], dt.bfloat16, tag="tinv1")
            nc.sync.dma_start(tinv_0[:], tinv_d[0])
            nc.sync.dma_start(tinv_1[:], tinv_d[1])
            e2_0 = cpool.tile([128, 512], dt.bfloat16, tag="e2_0")
            e2_1 = cpool.tile([128, 512], dt.bfloat16, tag="e2_1")
            nc.sync.dma_start(e2_0[:], e2_d[0])
            nc.sync.dma_start(e2_1[:], e2_d[1])
            tinv = [tinv_0, tinv_1]
            e2t = [e2_0, e2_1]
            w2 = [w2_0, w2_1]

            slots = {}

            def front(i):
                b, e = divmod(i, E)
                rec = recpool.tile([128, RECW], dt.bfloat16, tag="rec")
                nc.sync.dma_start(rec[:], rec_d[b, e])
                p1 = p1pool.tile([128, 512], dt.float32, tag="p1")
                nc.tensor.matmul(p1[:], rec[:, 0:128], w2[0][:], start=True, stop=False)
                nc.tensor.matmul(p1[:], rec[:, 128:256], w2[1][:], start=False, stop=True)
                slots[i] = (rec, p1)

            def back(i):
                b, e = divmod(i, E)
                rec, p1 = slots.pop(i)
                p1sb = pool.tile([128, 512], dt.bfloat16, tag="p1sb")
                nc.scalar.copy(p1sb[:], p1[:])
                m1 = pool.tile([128, 512], dt.bfloat16, tag="m1")
                m2 = pool.tile([128, 512], dt.bfloat16, tag="m2")
                uv = pool.tile([128, 512], dt.bfloat16, tag="uv")
                # m1 = [p1re*Cre | p1im*Cim], m2 = [p1re*Cim | p1im*Cre]
                nc.vector.tensor_mul(m1[:], p1sb[:], rec[:, 256:768])
                nc.vector.tensor_mul(m2[:], p1sb[:], rec[:, 512:1024])
                nc.vector.tensor_sub(uv[:, 0:256], m1[:, 0:256], m1[:, 256:512])
                nc.gpsimd.tensor_add(uv[:, 256:512], m2[:, 0:256], m2[:, 256:512])
                if e == 0:
                    slots[("pX", b)] = pxpool.tile([K1, 512], dt.float32, tag="pX",
                                                   name="pX")
                pX = slots[("pX", b)]
                # One accumulation group per PSUM bank: start only on the very
                # first matmul (start marks the whole 2KB zero region), stop on
                # the very last.
                st = e == 0
                sp = e == E - 1
                nc.tensor.matmul(pX[:, 0:256], rec[:, 1024:1089], uv[:, 0:256],
                                 start=st, stop=False)
                nc.tensor.matmul(pX[:, 0:256], rec[:, 1154:1219], uv[:, 256:512],
                                 start=False, stop=False)
                nc.tensor.matmul(pX[:, 256:512], rec[:, 1024:1089], uv[:, 256:512],
                                 start=False, stop=False)
                nc.tensor.matmul(pX[:, 256:512], rec[:, 1089:1154], uv[:, 0:256],
                                 start=False, stop=sp)
                if e == E - 1:
                    inverse(b, slots.pop(("pX", b)))

            def inverse(b, pX):
                xsb = ipool.tile([K1, 512], dt.bfloat16, tag="xsb")
                nc.scalar.copy(xsb[:], pX[:])
                nc.gpsimd.dma_start(aux_d[b, 0:1], xsb[0:1, 0:1])
                nc.gpsimd.dma_start(aux_d[b, 1:2], xsb[64:65, 0:1])
                pG = pgpool.tile([128, 512], dt.float32, tag="pG", name="pG")
                for c in range(2):
                    xre = xsb[:, c * 128:(c + 1) * 128]
                    xim = xsb[:, 256 + c * 128:256 + (c + 1) * 128]
                    o = c * 256
                    nc.tensor.matmul(pG[:, o:o + 128], xre, e1[:, 0:128],
                                     start=(c == 0), stop=False)
                    nc.tensor.matmul(pG[:, o:o + 128], xim, e1[:, 256:384],
                                     start=False, stop=False)
                    nc.tensor.matmul(pG[:, o + 128:o + 256], xre, e1[:, 128:256],
                                     start=False, stop=False)
                    nc.tensor.matmul(pG[:, o + 128:o + 256], xim, e1[:, 0:128],
                                     start=False, stop=(c == 1))
                gts = []
                for c in range(2):
                    gsb = ipool.tile([128, 256], dt.bfloat16, tag=f"gsb{c}")
                    nc.scalar.copy(gsb[:], pG[:, c * 256:(c + 1) * 256])
                    g1 = ipool.tile([128, 128], dt.bfloat16, tag=f"g1{c}")
                    g2 = ipool.tile([128, 128], dt.bfloat16, tag=f"g2{c}")
                    g3 = ipool.tile([128, 128], dt.bfloat16, tag=f"g3{c}")
                    g4 = ipool.tile([128, 128], dt.bfloat16, tag=f"g4{c}")
                    gt = ipool.tile([128, 256], dt.bfloat16, tag=f"gt{c}")
                    nc.vector.tensor_mul(g1[:], gsb[:, 0:128], tinv[c][:, 0:128])
                    nc.vector.tensor_mul(g2[:], gsb[:, 128:256], tinv[c][:, 128:256])
                    nc.vector.tensor_sub(gt[:, 0:128], g1[:], g2[:])
                    nc.gpsimd.tensor_mul(g3[:], gsb[:, 0:128], tinv[c][:, 128:256])
                    nc.gpsimd.tensor_mul(g4[:], gsb[:, 128:256], tinv[c][:, 0:128])
                    nc.vector.tensor_add(gt[:, 128:256], g3[:], g4[:])
                    gts.append(gt)
                pY = pypool.tile([128, 512], dt.float32, tag="pY", name="pY")
                for j in range(2):
                    nc.tensor.matmul(pY[:, j * 128:(j + 1) * 128],
                                     e2t[0][:, j * 128:(j + 1) * 128],
                                     gts[0][:, 0:128], start=(j == 0), stop=False)
                    nc.tensor.matmul(pY[:, j * 128:(j + 1) * 128],
                                     e2t[0][:, 256 + j * 128:256 + (j + 1) * 128],
                                     gts[0][:, 128:256], start=False, stop=False)
                    nc.tensor.matmul(pY[:, j * 128:(j + 1) * 128],
                                     e2t[1][:, j * 128:(j + 1) * 128],
                                     gts[1][:, 0:128], start=False, stop=False)
                    nc.tensor.matmul(pY[:, j * 128:(j + 1) * 128],
                                     e2t[1][:, 256 + j * 128:256 + (j + 1) * 128],
                                     gts[1][:, 128:256], start=False,
                                     stop=(j == 1))
                for j in range(2):
                    ysb = ipool.tile([128, 128], dt.float32, tag=f"ysb{j}")
                    nc.scalar.copy(ysb[:], pY[:, j * 128:(j + 1) * 128])
                    nc.sync.dma_start(out_d[b, j * 128:(j + 1) * 128, :], ysb[:])

            for i in range(S + LAG):
                if i < S:
                    front(i)
                if i >= LAG:
                    back(i - LAG)

    nc.compile()
    return nc


def kernel(time_latent, stems, targets, W_pos, b_pos):
    from concourse.bass_utils import run_bass_kernel_spmd

    # host: positions (tiny linear+sigmoid, fp32 exactly like the reference)
    z = np.einsum("bed,od->beo", time_latent.astype(F32), W_pos.astype(F32))
    z = z.reshape(B, E) + b_pos.reshape(1)[0]
    pos = 1.0 / (1.0 + np.exp(-z, dtype=F32))
    s = (pos * np.float32(N)).astype(np.float64)

    W2cat, e1cat, tinv, e2 = _host_consts()
    n1 = np.arange(N1)
    k2 = np.arange(N2)
    k1 = np.arange(K1)
    T = np.exp(-2j * np.pi * np.outer(n1, k2) / N)   # (n1, k2)
    W1 = np.exp(-2j * np.pi * np.outer(n1, k1) / N1)  # (n1, k1)

    w2cat_b = W2cat.astype(BF16)
    e1cat_b = e1cat.astype(BF16)
    tinv_b = tinv.astype(BF16)
    e2_b = e2.astype(BF16)

    nc = _build_graph()
    in_maps = []
    for c in range(NCORES):
        sl = slice(c * BC, (c + 1) * BC)
        s_flat = s[sl].reshape(-1)                                   # (S,)
        rec = np.empty((S, 128, RECW), dtype=BF16)
        # stems: (S, 256, 128) -> (S, 2, 128, 128) -> (S, 128, 2, 128)
        st = stems[sl].reshape(S, 2, 128, 128).transpose(0, 2, 1, 3)
        rec[:, :, 0:256] = st.reshape(S, 128, 256).astype(BF16)
        A = np.exp(-2j * np.pi * np.outer(s_flat, k2) / N)           # (S, k2)
        C = T[None, :, :] * A[:, None, :]                            # (S, n1, k2)
        cre = C.real.astype(BF16)
        rec[:, :, 256:512] = cre
        rec[:, :, 512:768] = C.imag.astype(BF16)
        rec[:, :, 768:1024] = cre
        del C, cre
        Bt = np.exp(-2j * np.pi * np.outer(s_flat, k1) / N1)         # (S, k1)
        M = W1[None, :, :] * Bt[:, None, :]                          # (S, n1, k1)
        rec[:, :, 1024:1089] = M.real.astype(BF16)
        rec[:, :, 1089:1154] = M.imag.astype(BF16)
        rec[:, :, 1154:1219] = (-M.imag).astype(BF16)
        rec[:, :, 1219:1220] = 0
        del M
        in_maps.append({
            "rec": rec.reshape(BC, E, 128, RECW),
            "w2cat": w2cat_b,
            "e1cat": e1cat_b,
            "tinv": tinv_b,
            "e2": e2_b,
        })

    import os
    trace = bool(int(os.environ.get("ATHENA_TRACE", "0")))
    res = run_bass_kernel_spmd(nc, in_maps, core_ids=list(range(NCORES)), trace=trace)
    if trace:
        print(f"HW exec time: {res.exec_time_ns} ns")
    outs = []
    sign = np.where(np.arange(N) % 2 == 0, 1.0, -1.0).astype(F32)
    for c in range(NCORES):
        y = res.results[c]["out"].reshape(BC, N).astype(F32)
        aux = res.results[c]["aux"].astype(F32)          # (BC, 2) = X0, XNyq
        y = y + (-aux[:, 0:1] + sign[None, :] * aux[:, 1:2]) / np.float32(N)
        outs.append(y)
    return np.concatenate(outs, 0).reshape(B, 1, N).astype(F32)
